# revision 24
# baseline (speedup 1.0000x reference)
"""Trainium2 Bass kernel for DeformableAttention — v2.

Contract: kernel(**inputs) takes FULL unsharded fp32 inputs (B=16) and
returns the FULL output [16, 2048, 256] fp32. Internally shards the batch
across 8 NeuronCores (2 batches per core), builds+runs one SPMD Bass
program via run_bass_kernel_spmd.

v2 vs baseline:
  - Stage 3 accumulates value tiles for a chunk of a level in SBUF
    (v_acc, double-buffered) and writes the 4-slot patch table with ONE
    dma_start per (chunk, slot, head), padded to full 128-row tiles
    (rows past the level end hold stale-but-finite data that is only
    read with zero weight or later overwritten). Slot-write dst APs are
    enumerated p-major so the SBUF source keeps its partition dim first
    (partition-middle DMA APs silently corrupt on HW).
  - Combine: bf16 multiply alternates between Pool (gpsimd) and DVE per
    query block; one XY-axis reduce per (qb, head-pair) with 2 heads
    batched; A accumulates in bf16 via a small f32 scratch.
  - Gathers split 4-way (4 query blocks each), issued one group ahead of
    the combines, and the two batches' gather/combine phases interleave
    so one batch's engine work hides the other's gather DMA.
  - Slot writes are emitted one chunk late so the next chunk's enc loads
    issue before the wait-heavy slot DMAs (in-order SP sequencer).
"""
import sys

sys.path.insert(0, "/opt/trn_rl_repo")

import numpy as np
import ml_dtypes

import concourse.bass as bass
import concourse.mybir as mybir
import concourse.tile as tile
from concourse import bacc
from concourse.bass import ds
from concourse.masks import make_identity

F32 = mybir.dt.float32
BF16 = mybir.dt.bfloat16
FP16 = mybir.dt.float16
I32 = mybir.dt.int32
OP = mybir.AluOpType
AF = mybir.ActivationFunctionType
AX = mybir.AxisListType

NCORES = 8
B_FULL = 16
BPC = B_FULL // NCORES  # 2 batches per core
Lq, Lv, D, H, HD, L, P = 2048, 13125, 256, 8, 32, 3, 4
J = L * P  # 12
NQB = Lq // 128  # 16
SHAPES = [(100, 100), (50, 50), (25, 25)]
LVBASE = [0, 10000, 12500]
PAD = 104  # zero-pad rows before each level segment (> w_max + 2)
LVSTART = [PAD, PAD + 10000 + PAD, PAD + 10000 + PAD + 2500 + PAD]
NR = LVSTART[2] + 625 + 27  # patch-table rows per (b,h)
LV_TILES = [79, 20, 5]        # ceil(h*w/128) per level
# stage-3 chunks: (level, first tile, n tiles); v_acc is double-buffered
# across chunks so slot-write DMAs overlap the next chunk's matmuls
V_CHUNKS = [(0, 0, 20), (0, 20, 20), (0, 40, 20), (0, 60, 19),
            (1, 0, 20), (2, 0, 5)]
NT = 20                       # v_acc tiles per buffer


def _build_program():
    nc = bacc.Bacc(
        "TRN2", target_bir_lowering=False, debug=False, num_devices=NCORES
    )

    # ---- DRAM I/O (enc bf16, x fp16 to cut per-iter transfer; bf16 x
    # would cost 0.015 rel err via coordinate perturbation, fp16 ~0.002) ----
    x_d = nc.dram_tensor("x", (BPC, Lq, D), FP16, kind="ExternalInput").ap()
    enc_d = nc.dram_tensor("enc", (BPC, Lv, D), BF16, kind="ExternalInput").ap()
    wq_d = nc.dram_tensor("wq", (128, 2, D), F32, kind="ExternalInput").ap()
    wcat_d = nc.dram_tensor("wcat", (128, 2, 290), F32, kind="ExternalInput").ap()
    wv_d = nc.dram_tensor("wv", (128, 2, D), BF16, kind="ExternalInput").ap()
    wout_d = nc.dram_tensor("wout", (128, 2, D), BF16, kind="ExternalInput").ap()
    bias_d = nc.dram_tensor("bias_rep", (128, 192), F32, kind="ExternalInput").ap()
    cs_d = nc.dram_tensor("cs24", (128, 24), F32, kind="ExternalInput").ap()
    dmax_d = nc.dram_tensor("dmax24", (128, 24), F32, kind="ExternalInput").ap()
    dmaxm1_d = nc.dram_tensor("dmaxm1", (128, 24), F32, kind="ExternalInput").ap()
    wmul_d = nc.dram_tensor("wmul12", (128, 12), F32, kind="ExternalInput").ap()
    lpb_d = nc.dram_tensor("lpb12", (128, 12), F32, kind="ExternalInput").ap()
    out_d = nc.dram_tensor("out", (BPC, Lq, D), BF16, kind="ExternalOutput").ap()
    vp_d = [
        nc.dram_tensor(f"vp{b}", (H, NR, 128), BF16, kind="Internal").ap()
        for b in range(BPC)
    ]

    with tile.TileContext(nc) as tc:
        _body(
            nc, tc, x_d, enc_d, wq_d, wcat_d, wv_d, wout_d, bias_d,
            cs_d, dmax_d, dmaxm1_d, wmul_d, lpb_d, out_d, vp_d,
        )
    nc.compile()
    return nc


def _body(nc, tc, x_d, enc_d, wq_d, wcat_d, wv_d, wout_d, bias_d,
          cs_d, dmax_d, dmaxm1_d, wmul_d, lpb_d, out_d, vp_d):
    from contextlib import ExitStack

    tt = nc.vector.tensor_tensor
    tsc = nc.vector.tensor_scalar
    DVE_COPY = nc.vector.tensor_copy
    ACOPY = nc.scalar.copy

    ctx = ExitStack()
    cpool = ctx.enter_context(tc.tile_pool(name="consts", bufs=1))
    wq = cpool.tile([128, 2, D], F32, tag="wq")
    wcat = cpool.tile([128, 2, 290], F32, tag="wcat")
    wv = cpool.tile([128, 2, D], BF16, tag="wv")
    wout = cpool.tile([128, 2, D], BF16, tag="wout")
    bias = cpool.tile([128, 192], F32, tag="bias")
    cs = cpool.tile([128, 24], F32, tag="cs")
    dmax = cpool.tile([128, 24], F32, tag="dmax")
    dmaxm1 = cpool.tile([128, 24], F32, tag="dmaxm1")
    wmul = cpool.tile([128, 12], F32, tag="wmul")
    lpb = cpool.tile([128, 12], F32, tag="lpb")
    ident = cpool.tile([128, 128], F32, tag="ident")
    identb = cpool.tile([128, 128], BF16, tag="identb")
    identh = cpool.tile([128, 128], FP16, tag="identh")
    zt = cpool.tile([128, H // 2, 128], BF16, tag="zt")

    for t, d in ((wq, wq_d), (wcat, wcat_d), (wv, wv_d), (wout, wout_d),
                 (bias, bias_d), (cs, cs_d), (dmax, dmax_d),
                 (dmaxm1, dmaxm1_d), (wmul, wmul_d), (lpb, lpb_d)):
        nc.sync.dma_start(t[...], d)
    make_identity(nc, ident[...])
    nc.vector.tensor_copy(identb[...], ident[...])
    nc.vector.tensor_copy(identh[...], ident[...])
    nc.gpsimd.memset(zt[...], 0.0)

    bpool = ctx.enter_context(tc.tile_pool(name="perb", bufs=1))
    psum = ctx.enter_context(tc.tile_pool(name="psum", bufs=2, space="PSUM"))
    # NOTE: "mm" tiles get bufs=1 via tile() kwarg
    psumt = ctx.enter_context(tc.tile_pool(name="psumt", bufs=2, space="PSUM"))
    work = ctx.enter_context(tc.tile_pool(name="work", bufs=2))

    # zero strips: same geometry as baseline (lead pad + level tails)
    zero_strips = [(0, PAD)]
    for lv, (hh, ww) in enumerate(SHAPES):
        t0 = LVSTART[lv] + hh * ww - (ww + 2)
        t1 = LVSTART[lv] + hh * ww + (PAD if lv < 2 else 27)
        zero_strips.append((t0, t1))

    for b in range(BPC):
        for h0 in (0, H // 2):
            for (r0, r1) in zero_strips:
                r = r0
                while r < r1:
                    n = min(128, r1 - r)
                    nc.sync.dma_start(
                        vp_d[b][ds(h0, H // 2), ds(r, n), :]
                        .rearrange("h p c -> p h c"),
                        zt[:n, :, :],
                    )
                    r += n

    # persistent per-b tiles (double-tagged where consumed downstream)
    refpix = bpool.tile([128, NQB, 24], F32, tag="refpix")
    A = bpool.tile([128, NQB, D], BF16, tag="A")
    off_b = [bpool.tile([128, NQB, 192], F32, tag=f"off{b}",
                        name=f"off_{b}") for b in range(BPC)]
    refs_b = [bpool.tile([128, NQB, 2], F32, tag=f"refs{b}",
                         name=f"refs_{b}") for b in range(BPC)]
    attn_b = [bpool.tile([128, NQB, H, 12], BF16, tag=f"attn{b}",
                         name=f"attn_{b}") for b in range(BPC)]
    W4_b = [bpool.tile([128, NQB, H, J, 4], BF16, tag=f"W4{b}",
                       name=f"W4_{b}") for b in range(BPC)]
    idx_b = [bpool.tile([128, H, NQB, J], F32, tag=f"idx{b}",
                        name=f"idx_{b}") for b in range(BPC)]

    def stage12(b):
        # ---- stage 1+2 (chunked): xT/qT per 512 queries, then proj ----
        off_sb, refs, attn = off_b[b], refs_b[b], attn_b[b]
        for cn in range(4):
            xT = work.tile([128, 2, 512], F32, tag="xq", bufs=2, name="xT")
            xt = work.tile([128, 4, D], FP16, tag="xtile", bufs=1)
            nc.sync.dma_start(
                xt[...],
                x_d[b, ds(cn * 512, 512), :].rearrange(
                    "(s p) c -> p s c", p=128),
            )
            for sq in range(4):
                pt = psumt.tile([128, 2, 128], FP16, tag="tp")
                for k in range(2):
                    nc.tensor.transpose(
                        pt[:, k, :], xt[:, sq, ds(k * 128, 128)], identh[...]
                    )
                ACOPY(xT[:, :, ds(sq * 128, 128)], pt[...])
            qT = work.tile([128, 2, 512], F32, tag="xq", bufs=2, name="qT")
            for m in range(2):
                pq = psum.tile([128, 512], F32, tag="mm", bufs=1)
                for k in range(2):
                    nc.tensor.matmul(
                        pq[...],
                        wq[:, k, ds(m * 128, 128)],
                        xT[:, k, :],
                        start=(k == 0), stop=(k == 1),
                    )
                ACOPY(qT[:, m, :], pq[...])
            for sq in range(4):
                qb = cn * 4 + sq
                pp = psum.tile([128, 290], F32, tag="mm", bufs=1)
                for k in range(2):
                    nc.tensor.matmul(
                        pp[...], qT[:, k, ds(sq * 128, 128)], wcat[:, k, :],
                        start=(k == 0), stop=(k == 1),
                    )
                tt(off_sb[:, qb, :], pp[:, 0:192], bias[...], OP.add)
                nc.scalar.activation(refs[:, qb, :], pp[:, 192:194], AF.Sigmoid)
                ex = work.tile([128, 96], F32, tag="ex", bufs=1)
                nc.scalar.activation(ex[...], pp[:, 194:290], AF.Exp)
                sm = work.tile([128, 8], F32, tag="sm")
                nc.vector.tensor_reduce(
                    sm[...], ex.rearrange("p (h j) -> p h j", j=12), AX.X, OP.add
                )
                nc.vector.reciprocal(sm[...], sm[...])
                tt(
                    attn[:, qb, :, :],
                    ex.rearrange("p (h j) -> p h j", j=12),
                    sm[:, :, None].broadcast_to((128, 8, 12)),
                    OP.mult,
                )

    def stage3(b):
        # ---- stage 3: value matmul into SBUF v_acc, then slot writes ----
        # v_acc holds one chunk of a level at a time (double-buffered so
        # the next chunk's matmuls overlap this chunk's slot-write DMAs);
        # per chunk: matmuls fill v_acc tiles, then one dma per
        # (slot, head) full-tile span (+ tail call on the last chunk).
        pending_slots = []

        def flush_slots():
            for fn in pending_slots:
                fn()
            pending_slots.clear()

        for lv, ct0, cnt in V_CHUNKS:
            v_acc = work.tile([128, NT, D], BF16, tag="vacc", bufs=2,
                              name="vacc")
            hh_, ww_ = SHAPES[lv]
            npos = hh_ * ww_
            for t2 in range(ct0, ct0 + cnt, 2):
                nt2 = min(2, ct0 + cnt - t2)
                npair = min(nt2 * 128, npos - t2 * 128)
                et = work.tile([128, 2, D], BF16, tag="etile")
                if npair == nt2 * 128:
                    # p-major src enumeration keeps partition dim first
                    nc.sync.dma_start(
                        et[:, 0:nt2, :],
                        enc_d[b, ds(LVBASE[lv] + t2 * 128, nt2 * 128), :]
                        .rearrange("(t p) c -> p t c", p=128),
                    )
                else:
                    for t in range(t2, t2 + nt2):
                        n = min(128, npos - t * 128)
                        nc.sync.dma_start(
                            et[0:n, t - t2, :],
                            enc_d[b, ds(LVBASE[lv] + t * 128, n), :],
                        )
                for t in range(t2, t2 + nt2):
                    n = min(128, npos - t * 128)
                    pt = psumt.tile([128, 2, 128], BF16, tag="tp")
                    for k in range(2):
                        nc.tensor.transpose(
                            pt[:, k, :n], et[:n, t - t2, ds(k * 128, 128)],
                            identb[:n, :n]
                        )
                    etT = work.tile([128, 2, 128], BF16, tag="etT", bufs=1)
                    ACOPY(etT[...], pt[...])
                    pv = psum.tile([128, D], F32, tag="pv")
                    for k in range(2):
                        nc.tensor.matmul(
                            pv[:n, :],
                            etT[:, k, :n],
                            wv[:, k, :],
                            start=(k == 0), stop=(k == 1),
                        )
                    ACOPY(v_acc[:n, t - ct0, :], pv[:n, :])
            # full-tile padded span: rows past the level end hold stale
            # finite v_acc data; they are only ever read with zero weight
            # (or overwritten by the next level's writes, issued later).
            def emit_slots(lv=lv, ct0=ct0, cnt=cnt, ww_=ww_, v_acc=v_acc):
                vsrc = v_acc.rearrange("p t (h c) -> p t h c", c=HD)
                for sl, dlt in enumerate((0, 1, ww_, ww_ + 1)):
                    r0 = LVSTART[lv] - dlt + ct0 * 128
                    for hI in range(H):
                        nc.sync.dma_start(
                            vp_d[b][hI, ds(r0, cnt * 128), ds(sl * HD, HD)]
                            .rearrange("(t p) c -> p t c", p=128),
                            vsrc[:, ds(0, cnt), hI, :],
                        )
            flush_slots()
            pending_slots.append(emit_slots)
        flush_slots()

    def stage4(b):
        W4 = W4_b[b]
        idx = idx_b[b]
        off_sb, refs, attn = off_b[b], refs_b[b], attn_b[b]
        # ---- stage 4: coords + weights + indices (same math as baseline) --
        nc.vector.tensor_tensor(
            refpix.rearrange("p q (j c) -> p q j c", c=2),
            refs[:, :, None, :].broadcast_to((128, NQB, 12, 2)),
            cs.rearrange("p (j c) -> p j c", c=2)[:, None, :, :].broadcast_to(
                (128, NQB, 12, 2)
            ),
            OP.mult,
        )
        nc.vector.tensor_scalar(refpix[...], refpix[...], -0.5, None, OP.add)

        NH = 2  # heads per coord-group
        NQC = NQB // 8  # query blocks per stage-4 chunk
        for hg in range(H // NH):
          for q0 in range(0, NQB, NQC):
            # comparison ALU ops (is_gt/is_ge/...) are DVE-only in walrus
            # codegen, so stage 4 stays on DVE
            tt = nc.vector.tensor_tensor
            tsc = nc.vector.tensor_scalar
            DVE_COPY = nc.vector.tensor_copy
            hs = hg * NH
            shp = (128, NQC, NH, J, 2)
            nel = NQC * NH * J * 2
            s0 = work.tile([128, nel], F32, tag="cs0", bufs=2)
            s2 = work.tile([128, nel], F32, tag="cs2", bufs=2)
            s3 = work.tile([128, nel], F32, tag="cs3", bufs=2)
            s4 = work.tile([128, nel], F32, tag="cs4", bufs=2)
            s5 = work.tile([128, nel], F32, tag="cs5", bufs=2)
            ti = work.tile([128, nel], I32, tag="cti", bufs=2)
            v0 = lambda t: t.rearrange("p (q h j c) -> p q h j c", q=NQC, h=NH, j=J)
            csb = cs.rearrange("p (j c) -> p j c", c=2)[:, None, None, :, :].broadcast_to(shp)
            dmaxb = dmax.rearrange("p (j c) -> p j c", c=2)[:, None, None, :, :].broadcast_to(shp)
            dmaxm1b = dmaxm1.rearrange("p (j c) -> p j c", c=2)[:, None, None, :, :].broadcast_to(shp)
            offv = off_sb.rearrange("p q (h j c) -> p q h j c", h=H, c=2)[:, ds(q0, NQC), ds(hs, NH), :, :]
            tt(v0(s0), offv, csb, OP.mult)
            tt(
                v0(s0), v0(s0),
                refpix.rearrange("p q (j c) -> p q j c", c=2)[:, ds(q0, NQC), None, :, :]
                .broadcast_to(shp),
                OP.add,
            )
            DVE_COPY(ti[...], s0[...])
            DVE_COPY(s2[...], ti[...])
            tt(s3[...], s2[...], s0[...], OP.is_gt)
            tt(s2[...], s2[...], s3[...], OP.subtract)
            tt(s0[...], s0[...], s2[...], OP.subtract)
            tsc(s3[...], s2[...], 0.0, None, OP.is_ge)
            tt(v0(s4), v0(s2), dmaxb, OP.is_le)
            tt(s3[...], s3[...], s4[...], OP.mult)
            tt(v0(s5), v0(s2), dmaxm1b, OP.is_le)
            tsc(s4[...], s2[...], -1.0, None, OP.is_ge)
            tt(s4[...], s4[...], s5[...], OP.mult)
            tsc(s5[...], s0[...], -1.0, 1.0, OP.mult, OP.add)
            tt(s3[...], s5[...], s3[...], OP.mult)
            tt(s4[...], s0[...], s4[...], OP.mult)
            xslice = lambda t: v0(t)[:, :, :, :, 0]
            yslice = lambda t: v0(t)[:, :, :, :, 1]
            wyT = work.tile([128, nel // 2], F32, tag="wyT", bufs=2)
            wyB = work.tile([128, nel // 2], F32, tag="wyB", bufs=2)
            v1 = lambda t: t.rearrange("p (q h j) -> p q h j", q=NQC, h=NH)
            attv = attn[:, ds(q0, NQC), ds(hs, NH), :]
            tt(v1(wyT), yslice(s3), attv, OP.mult)
            tt(v1(wyB), yslice(s4), attv, OP.mult)
            w4v = W4[:, ds(q0, NQC), ds(hs, NH), :, :]
            tt(w4v[:, :, :, :, 0], v1(wyT), xslice(s3), OP.mult)
            tt(w4v[:, :, :, :, 1], v1(wyT), xslice(s4), OP.mult)
            tt(w4v[:, :, :, :, 2], v1(wyB), xslice(s3), OP.mult)
            tt(w4v[:, :, :, :, 3], v1(wyB), xslice(s4), OP.mult)
            tsc(s3[...], s2[...], -1.0, None, OP.max)
            tt(v0(s3), v0(s3), dmaxb, OP.min)
            wmulb = wmul[:, None, None, :].broadcast_to((128, NQC, NH, J))
            lpbb = lpb[:, None, None, :].broadcast_to((128, NQC, NH, J))
            pT = wyT  # reuse buffer
            tt(v1(pT), yslice(s3), wmulb, OP.mult)
            tt(v1(pT), v1(pT), xslice(s3), OP.add)
            tt(v1(pT), v1(pT), lpbb, OP.add)
            for hh in range(NH):
                DVE_COPY(idx[:, hs + hh, ds(q0, NQC), :], v1(pT)[:, :, hh, :])

    A_b = [A, bpool.tile([128, NQB, D], BF16, tag="A1", name="A_1")]

    def fold(b, hp):
        # fold this head pair's indices into wrapped int16
        idx = idx_b[b]
        w16_h = []
        for h2 in range(2):
            h = hp * 2 + h2
            idxw = work.tile([128, 768], I32, tag="idxw", bufs=4)
            w16 = idxw.bitcast(mybir.dt.int16)  # [128, 1536]
            Xh = idx[:, h, :, :].rearrange("p q j -> p (q j)")
            Ysb = work.tile([128, 2, 128], F32, tag="Ysb", bufs=1)
            for c in range(2):
                ptr = psumt.tile([128, 128], F32, tag="tpf")
                nc.tensor.transpose(
                    ptr[:96, :], Xh[:, ds(c * 96, 96)], ident[...]
                )
                ACOPY(Ysb[:96, c, :], ptr[:96, :])
            for qq in range(8):
                for c in range(2):
                    ptr2 = psumt.tile([128, 128], F32, tag="tpf")
                    nc.tensor.transpose(
                        ptr2[:16, :96],
                        Ysb[:96, c, ds(qq * 16, 16)],
                        ident[:96, :96],
                    )
                    # split 96 cols into 2 blocks of 48 (4-qb groups)
                    o = c * 768 + qq
                    ACOPY(w16[0:16, o:o + 377:8], ptr2[:16, 0:48])
                    ACOPY(w16[0:16, o + 384:o + 384 + 377:8],
                          ptr2[:16, 48:96])
            # replicate wrapped block to the other 7 core blocks
            # (log2 doubling: 16->32->64->128 partitions)
            for rep in (16, 32, 64):
                nc.sync.dma_start(
                    w16[ds(rep, rep), :], w16[ds(0, rep), :]
                )
            w16_h.append(w16)
        return w16_h

    def issue_gathers(b, hp, qg, w16_h):
        G = work.tile([128, 2, 48, 128], BF16, tag="G", name="G")
        for h2 in range(2):
            h = hp * 2 + h2
            nc.gpsimd.dma_gather(
                G[:, h2, :, :],
                vp_d[b][h, :, :],
                w16_h[h2][:, ds(qg * 384, 384)],
                num_idxs=4 * J * 128,
                num_idxs_reg=4 * J * 128,
                elem_size=128,
                single_packet=False,
            )
        return G

    def combines(b, hp, qg, G):
        # one mult+reduce per head covering all 4 query blocks of the
        # gather group; engines alternate DVE/Pool per (b, qg, head)
        W4 = W4_b[b]
        for h2 in range(2):
            h = hp * 2 + h2
            eng = nc.gpsimd if (b + qg + h2) % 2 == 0 else nc.vector
            # tag "vacc" reuses stage-3's buffers (dead once combines run)
            Pm = work.tile([128, 4, J, 4, HD], BF16, tag="vacc", name="Pm")
            eng.tensor_tensor(
                Pm[...],
                G[:, h2, :, :].rearrange(
                    "p (q j) (sl e) -> p q j sl e", j=J, e=HD
                ),
                W4[:, ds(qg * 4, 4), h, :, :][:, :, :, :, None]
                .broadcast_to((128, 4, J, 4, HD)),
                OP.mult,
            )
            Ared = work.tile([128, 4, HD], F32, tag="Ared", bufs=2)
            nc.vector.tensor_reduce(
                Ared[...],
                Pm.rearrange("p q j sl e -> p q e j sl"),
                AX.XY, OP.add,
            )
            ACOPY(
                A_b[b][:, ds(qg * 4, 4), ds(h * HD, HD)],
                Ared[...],
            )

    def stage56_pair():
        # ---- stage 5 both batches interleaved: one batch's combine fills
        # the other's gather latency ----
        for hp in range(4):
            w16_b = [fold(0, hp), fold(1, hp)]
            prev = None
            for qg in range(4):
                for bb in (0, 1):
                    Gn = issue_gathers(bb, hp, qg, w16_b[bb])
                    if prev is not None:
                        combines(*prev)
                    prev = (bb, hp, qg, Gn)
            combines(*prev)
    def stage6(b):
        for qb in range(NQB):
            pt = psumt.tile([128, 2, 128], BF16, tag="tpb", bufs=1)
            for k in range(2):
                nc.tensor.transpose(
                    pt[:, k, :], A_b[b][:, qb, ds(k * 128, 128)], identb[...]
                )
            AT = work.tile([128, 2, 128], BF16, tag="AT", bufs=1)
            ACOPY(AT[...], pt[...])
            po = psum.tile([128, D], F32, tag="pv")
            for k in range(2):
                nc.tensor.matmul(
                    po[...], AT[:, k, :], wout[:, k, :],
                    start=(k == 0), stop=(k == 1),
                )
            osb = work.tile([128, D], BF16, tag="osb", bufs=1)
            ACOPY(osb[...], po[...])
            nc.sync.dma_start(out_d[b, ds(qb * 128, 128), :], osb[...])

    # issue order: stage12 before stage3 so the projection matmuls aren't
    # queued behind stage 3's ~420 PE instructions (stage 4 DVE work then
    # overlaps stage 3's DMA/PE); b1's prep stages issue after b0's
    # combine so they fill PE/ACT/DMA while b0's gathers+reduces run.
    stage12(0)
    stage3(0); stage12(1); stage4(0)
    stage3(1); stage4(1)
    stage56_pair()
    stage6(0); stage6(1)
    ctx.close()


_CACHED = None


def _get_program():
    global _CACHED
    if _CACHED is None:
        _CACHED = _build_program()
    return _CACHED


def make_host_inputs(Wq, Wref, Woff, off_bias, Wattn, Wv, Wout):
    """Device-layout weight/constant arrays shared by all cores."""
    bf = ml_dtypes.bfloat16
    wcat = np.concatenate([Woff, Wref, Wattn], axis=1)  # [256, 290]

    def halves(w):
        return np.ascontiguousarray(w.reshape(2, 128, -1).transpose(1, 0, 2))

    rep = lambda v: np.ascontiguousarray(
        np.broadcast_to(np.asarray(v, np.float32)[None, :], (128, len(v)))
    )
    cs24, dmax24, dmaxm1 = [], [], []
    wmul12, lpb12 = [], []
    for l, (hh, ww) in enumerate(SHAPES):
        for p in range(P):
            cs24 += [float(ww), float(hh)]
            dmax24 += [float(ww - 1), float(hh - 1)]
            dmaxm1 += [float(ww - 2), float(hh - 2)]
            wmul12.append(float(ww))
            lpb12.append(float(LVSTART[l]))
    return {
        "wq": halves(Wq).astype(np.float32),
        "wcat": halves(wcat).astype(np.float32),
        "wv": halves(Wv).astype(bf),
        "wout": halves(Wout).astype(bf),
        "bias_rep": rep(off_bias.astype(np.float32)),
        "cs24": rep(cs24),
        "dmax24": rep(dmax24),
        "dmaxm1": rep(dmaxm1),
        "wmul12": rep(wmul12),
        "lpb12": rep(lpb12),
    }


def make_in_maps(x, encoder_input, host_w):
    bf = ml_dtypes.bfloat16
    x = np.asarray(x).astype(np.float16)
    enc = np.asarray(encoder_input).astype(bf)
    in_maps = []
    for c in range(NCORES):
        m = dict(host_w)
        m["x"] = np.ascontiguousarray(x[c * BPC:(c + 1) * BPC])
        m["enc"] = np.ascontiguousarray(enc[c * BPC:(c + 1) * BPC])
        in_maps.append(m)
    return in_maps


def kernel(x, encoder_input, Wq, Wref, Woff, off_bias, Wattn, Wv, Wout):
    from concourse import bass_utils

    nc = _get_program()
    host_w = make_host_inputs(
        np.asarray(Wq, np.float32), np.asarray(Wref, np.float32),
        np.asarray(Woff, np.float32), np.asarray(off_bias, np.float32),
        np.asarray(Wattn, np.float32), np.asarray(Wv, np.float32),
        np.asarray(Wout, np.float32),
    )
    in_maps = make_in_maps(x, encoder_input, host_w)
    res = bass_utils.run_bass_kernel_spmd(nc, in_maps, core_ids=list(range(NCORES)))
    return np.concatenate(
        [np.asarray(r["out"]).astype(np.float32) for r in res.results], axis=0
    )



# revision 26
# speedup vs baseline: 1.2685x; 1.2685x over previous
"""Trainium2 Bass kernel for DeformableAttention — v4.

Contract: kernel(**inputs) takes FULL unsharded fp32 inputs (B=16) and
returns the FULL output [16, 2048, 256] fp32. Internally shards the batch
across 8 NeuronCores (2 batches per core), builds+runs one SPMD Bass
program via run_bass_kernel_spmd.

v4 vs v2:
  - I/O bytes minimized (the axon PJRT path re-ships every operand per
    execute at ~12.5 GB/s): enc uploaded bf16 (it only feeds the bf16
    value matmul), x uploaded fp16 (bf16 x costs 0.015 rel err via
    coordinate perturbation, fp16 ~0.002), out returned bf16 and cast
    to f32 on host.
  - Combines batched: one mult+reduce per (head, gather-group) covering
    4 query blocks (3x fewer DVE/Pool ops); Pm reuses stage-3's v_acc
    buffers (dead by combine time) to fit SBUF.

v2 vs baseline:
  - Stage 3 accumulates value tiles for a chunk of a level in SBUF
    (v_acc, double-buffered) and writes the 4-slot patch table with ONE
    dma_start per (chunk, slot, head), padded to full 128-row tiles
    (rows past the level end hold stale-but-finite data that is only
    read with zero weight or later overwritten). Slot-write dst APs are
    enumerated p-major so the SBUF source keeps its partition dim first
    (partition-middle DMA APs silently corrupt on HW).
  - Combine: bf16 multiply alternates between Pool (gpsimd) and DVE per
    query block; one XY-axis reduce per (qb, head-pair) with 2 heads
    batched; A accumulates in bf16 via a small f32 scratch.
  - Gathers split 4-way (4 query blocks each), issued one group ahead of
    the combines, and the two batches' gather/combine phases interleave
    so one batch's engine work hides the other's gather DMA.
  - Slot writes are emitted one chunk late so the next chunk's enc loads
    issue before the wait-heavy slot DMAs (in-order SP sequencer).
"""
import sys

sys.path.insert(0, "/opt/trn_rl_repo")

import numpy as np
import ml_dtypes

import concourse.bass as bass
import concourse.mybir as mybir
import concourse.tile as tile
from concourse import bacc
from concourse.bass import ds
from concourse.masks import make_identity

F32 = mybir.dt.float32
BF16 = mybir.dt.bfloat16
FP16 = mybir.dt.float16
I32 = mybir.dt.int32
OP = mybir.AluOpType
AF = mybir.ActivationFunctionType
AX = mybir.AxisListType

NCORES = 8
B_FULL = 16
BPC = B_FULL // NCORES  # 2 batches per core
Lq, Lv, D, H, HD, L, P = 2048, 13125, 256, 8, 32, 3, 4
J = L * P  # 12
NQB = Lq // 128  # 16
SHAPES = [(100, 100), (50, 50), (25, 25)]
LVBASE = [0, 10000, 12500]
PAD = 104  # zero-pad rows before each level segment (> w_max + 2)
LVSTART = [PAD, PAD + 10000 + PAD, PAD + 10000 + PAD + 2500 + PAD]
NR = LVSTART[2] + 625 + 27  # patch-table rows per (b,h)
LV_TILES = [79, 20, 5]        # ceil(h*w/128) per level
# stage-3 chunks: (level, first tile, n tiles); v_acc is double-buffered
# across chunks so slot-write DMAs overlap the next chunk's matmuls
V_CHUNKS = [(0, 0, 20), (0, 20, 20), (0, 40, 20), (0, 60, 19),
            (1, 0, 20), (2, 0, 5)]
NT = 20                       # v_acc tiles per buffer


def _build_program():
    nc = bacc.Bacc(
        "TRN2", target_bir_lowering=False, debug=False, num_devices=NCORES
    )

    # ---- DRAM I/O (enc bf16, x fp16 to cut per-iter transfer; bf16 x
    # would cost 0.015 rel err via coordinate perturbation, fp16 ~0.002) ----
    x_d = nc.dram_tensor("x", (BPC, Lq, D), FP16, kind="ExternalInput").ap()
    enc_d = nc.dram_tensor("enc", (BPC, Lv, D), BF16, kind="ExternalInput").ap()
    wq_d = nc.dram_tensor("wq", (128, 2, D), F32, kind="ExternalInput").ap()
    wcat_d = nc.dram_tensor("wcat", (128, 2, 290), F32, kind="ExternalInput").ap()
    wv_d = nc.dram_tensor("wv", (128, 2, D), BF16, kind="ExternalInput").ap()
    wout_d = nc.dram_tensor("wout", (128, 2, D), BF16, kind="ExternalInput").ap()
    bias_d = nc.dram_tensor("bias_rep", (128, 192), F32, kind="ExternalInput").ap()
    cs_d = nc.dram_tensor("cs24", (128, 24), F32, kind="ExternalInput").ap()
    dmax_d = nc.dram_tensor("dmax24", (128, 24), F32, kind="ExternalInput").ap()
    dmaxm1_d = nc.dram_tensor("dmaxm1", (128, 24), F32, kind="ExternalInput").ap()
    wmul_d = nc.dram_tensor("wmul12", (128, 12), F32, kind="ExternalInput").ap()
    lpb_d = nc.dram_tensor("lpb12", (128, 12), F32, kind="ExternalInput").ap()
    out_d = nc.dram_tensor("out", (BPC, Lq, D), BF16, kind="ExternalOutput").ap()
    vp_d = [
        nc.dram_tensor(f"vp{b}", (H, NR, 128), BF16, kind="Internal").ap()
        for b in range(BPC)
    ]

    with tile.TileContext(nc) as tc:
        _body(
            nc, tc, x_d, enc_d, wq_d, wcat_d, wv_d, wout_d, bias_d,
            cs_d, dmax_d, dmaxm1_d, wmul_d, lpb_d, out_d, vp_d,
        )
    nc.compile()
    return nc


def _body(nc, tc, x_d, enc_d, wq_d, wcat_d, wv_d, wout_d, bias_d,
          cs_d, dmax_d, dmaxm1_d, wmul_d, lpb_d, out_d, vp_d):
    from contextlib import ExitStack

    tt = nc.vector.tensor_tensor
    tsc = nc.vector.tensor_scalar
    DVE_COPY = nc.vector.tensor_copy
    ACOPY = nc.scalar.copy

    ctx = ExitStack()
    cpool = ctx.enter_context(tc.tile_pool(name="consts", bufs=1))
    wq = cpool.tile([128, 2, D], F32, tag="wq")
    wcat = cpool.tile([128, 2, 290], F32, tag="wcat")
    wv = cpool.tile([128, 2, D], BF16, tag="wv")
    wout = cpool.tile([128, 2, D], BF16, tag="wout")
    bias = cpool.tile([128, 192], F32, tag="bias")
    cs = cpool.tile([128, 24], F32, tag="cs")
    dmax = cpool.tile([128, 24], F32, tag="dmax")
    dmaxm1 = cpool.tile([128, 24], F32, tag="dmaxm1")
    wmul = cpool.tile([128, 12], F32, tag="wmul")
    lpb = cpool.tile([128, 12], F32, tag="lpb")
    ident = cpool.tile([128, 128], F32, tag="ident")
    identb = cpool.tile([128, 128], BF16, tag="identb")
    identh = cpool.tile([128, 128], FP16, tag="identh")
    zt = cpool.tile([128, H // 2, 128], BF16, tag="zt")

    for t, d in ((wq, wq_d), (wcat, wcat_d), (wv, wv_d), (wout, wout_d),
                 (bias, bias_d), (cs, cs_d), (dmax, dmax_d),
                 (dmaxm1, dmaxm1_d), (wmul, wmul_d), (lpb, lpb_d)):
        nc.sync.dma_start(t[...], d)
    make_identity(nc, ident[...])
    nc.vector.tensor_copy(identb[...], ident[...])
    nc.vector.tensor_copy(identh[...], ident[...])
    nc.gpsimd.memset(zt[...], 0.0)

    bpool = ctx.enter_context(tc.tile_pool(name="perb", bufs=1))
    psum = ctx.enter_context(tc.tile_pool(name="psum", bufs=2, space="PSUM"))
    # NOTE: "mm" tiles get bufs=1 via tile() kwarg
    psumt = ctx.enter_context(tc.tile_pool(name="psumt", bufs=2, space="PSUM"))
    work = ctx.enter_context(tc.tile_pool(name="work", bufs=2))

    # zero strips: same geometry as baseline (lead pad + level tails)
    zero_strips = [(0, PAD)]
    for lv, (hh, ww) in enumerate(SHAPES):
        t0 = LVSTART[lv] + hh * ww - (ww + 2)
        t1 = LVSTART[lv] + hh * ww + (PAD if lv < 2 else 27)
        zero_strips.append((t0, t1))

    for b in range(BPC):
        for h0 in (0, H // 2):
            for (r0, r1) in zero_strips:
                r = r0
                while r < r1:
                    n = min(128, r1 - r)
                    nc.sync.dma_start(
                        vp_d[b][ds(h0, H // 2), ds(r, n), :]
                        .rearrange("h p c -> p h c"),
                        zt[:n, :, :],
                    )
                    r += n

    # persistent per-b tiles (double-tagged where consumed downstream)
    refpix = bpool.tile([128, NQB, 24], F32, tag="refpix")
    A = bpool.tile([128, NQB, D], BF16, tag="A")
    off_b = [bpool.tile([128, NQB, 192], F32, tag=f"off{b}",
                        name=f"off_{b}") for b in range(BPC)]
    refs_b = [bpool.tile([128, NQB, 2], F32, tag=f"refs{b}",
                         name=f"refs_{b}") for b in range(BPC)]
    attn_b = [bpool.tile([128, NQB, H, 12], BF16, tag=f"attn{b}",
                         name=f"attn_{b}") for b in range(BPC)]
    W4_b = [bpool.tile([128, NQB, H, J, 4], BF16, tag=f"W4{b}",
                       name=f"W4_{b}") for b in range(BPC)]
    idx_b = [bpool.tile([128, H, NQB, J], F32, tag=f"idx{b}",
                        name=f"idx_{b}") for b in range(BPC)]

    def stage12(b):
        # ---- stage 1+2 (chunked): xT/qT per 512 queries, then proj ----
        off_sb, refs, attn = off_b[b], refs_b[b], attn_b[b]
        for cn in range(4):
            xT = work.tile([128, 2, 512], F32, tag="xq", bufs=2, name="xT")
            xt = work.tile([128, 4, D], FP16, tag="xtile", bufs=1)
            nc.sync.dma_start(
                xt[...],
                x_d[b, ds(cn * 512, 512), :].rearrange(
                    "(s p) c -> p s c", p=128),
            )
            for sq in range(4):
                pt = psumt.tile([128, 2, 128], FP16, tag="tp")
                for k in range(2):
                    nc.tensor.transpose(
                        pt[:, k, :], xt[:, sq, ds(k * 128, 128)], identh[...]
                    )
                ACOPY(xT[:, :, ds(sq * 128, 128)], pt[...])
            qT = work.tile([128, 2, 512], F32, tag="xq", bufs=2, name="qT")
            for m in range(2):
                pq = psum.tile([128, 512], F32, tag="mm", bufs=1)
                for k in range(2):
                    nc.tensor.matmul(
                        pq[...],
                        wq[:, k, ds(m * 128, 128)],
                        xT[:, k, :],
                        start=(k == 0), stop=(k == 1),
                    )
                ACOPY(qT[:, m, :], pq[...])
            for sq in range(4):
                qb = cn * 4 + sq
                pp = psum.tile([128, 290], F32, tag="mm", bufs=1)
                for k in range(2):
                    nc.tensor.matmul(
                        pp[...], qT[:, k, ds(sq * 128, 128)], wcat[:, k, :],
                        start=(k == 0), stop=(k == 1),
                    )
                tt(off_sb[:, qb, :], pp[:, 0:192], bias[...], OP.add)
                nc.scalar.activation(refs[:, qb, :], pp[:, 192:194], AF.Sigmoid)
                ex = work.tile([128, 96], F32, tag="ex", bufs=1)
                nc.scalar.activation(ex[...], pp[:, 194:290], AF.Exp)
                sm = work.tile([128, 8], F32, tag="sm")
                nc.vector.tensor_reduce(
                    sm[...], ex.rearrange("p (h j) -> p h j", j=12), AX.X, OP.add
                )
                nc.vector.reciprocal(sm[...], sm[...])
                tt(
                    attn[:, qb, :, :],
                    ex.rearrange("p (h j) -> p h j", j=12),
                    sm[:, :, None].broadcast_to((128, 8, 12)),
                    OP.mult,
                )

    def stage3(b):
        # ---- stage 3: value matmul into SBUF v_acc, then slot writes ----
        # v_acc holds one chunk of a level at a time (double-buffered so
        # the next chunk's matmuls overlap this chunk's slot-write DMAs);
        # per chunk: matmuls fill v_acc tiles, then one dma per
        # (slot, head) full-tile span (+ tail call on the last chunk).
        pending_slots = []

        def flush_slots():
            for fn in pending_slots:
                fn()
            pending_slots.clear()

        for lv, ct0, cnt in V_CHUNKS:
            v_acc = work.tile([128, NT, D], BF16, tag="vacc", bufs=2,
                              name="vacc")
            hh_, ww_ = SHAPES[lv]
            npos = hh_ * ww_
            for t2 in range(ct0, ct0 + cnt, 2):
                nt2 = min(2, ct0 + cnt - t2)
                npair = min(nt2 * 128, npos - t2 * 128)
                et = work.tile([128, 2, D], BF16, tag="etile")
                if npair == nt2 * 128:
                    # p-major src enumeration keeps partition dim first
                    nc.sync.dma_start(
                        et[:, 0:nt2, :],
                        enc_d[b, ds(LVBASE[lv] + t2 * 128, nt2 * 128), :]
                        .rearrange("(t p) c -> p t c", p=128),
                    )
                else:
                    for t in range(t2, t2 + nt2):
                        n = min(128, npos - t * 128)
                        nc.sync.dma_start(
                            et[0:n, t - t2, :],
                            enc_d[b, ds(LVBASE[lv] + t * 128, n), :],
                        )
                for t in range(t2, t2 + nt2):
                    n = min(128, npos - t * 128)
                    pt = psumt.tile([128, 2, 128], BF16, tag="tp")
                    for k in range(2):
                        nc.tensor.transpose(
                            pt[:, k, :n], et[:n, t - t2, ds(k * 128, 128)],
                            identb[:n, :n]
                        )
                    etT = work.tile([128, 2, 128], BF16, tag="etT", bufs=1)
                    ACOPY(etT[...], pt[...])
                    pv = psum.tile([128, D], F32, tag="pv")
                    for k in range(2):
                        nc.tensor.matmul(
                            pv[:n, :],
                            etT[:, k, :n],
                            wv[:, k, :],
                            start=(k == 0), stop=(k == 1),
                        )
                    ACOPY(v_acc[:n, t - ct0, :], pv[:n, :])
            # full-tile padded span: rows past the level end hold stale
            # finite v_acc data; they are only ever read with zero weight
            # (or overwritten by the next level's writes, issued later).
            def emit_slots(lv=lv, ct0=ct0, cnt=cnt, ww_=ww_, v_acc=v_acc):
                vsrc = v_acc.rearrange("p t (h c) -> p t h c", c=HD)
                for sl, dlt in enumerate((0, 1, ww_, ww_ + 1)):
                    r0 = LVSTART[lv] - dlt + ct0 * 128
                    for hI in range(H):
                        nc.sync.dma_start(
                            vp_d[b][hI, ds(r0, cnt * 128), ds(sl * HD, HD)]
                            .rearrange("(t p) c -> p t c", p=128),
                            vsrc[:, ds(0, cnt), hI, :],
                        )
            flush_slots()
            pending_slots.append(emit_slots)
        flush_slots()

    def stage4(b):
        W4 = W4_b[b]
        idx = idx_b[b]
        off_sb, refs, attn = off_b[b], refs_b[b], attn_b[b]
        # ---- stage 4: coords + weights + indices (same math as baseline) --
        nc.vector.tensor_tensor(
            refpix.rearrange("p q (j c) -> p q j c", c=2),
            refs[:, :, None, :].broadcast_to((128, NQB, 12, 2)),
            cs.rearrange("p (j c) -> p j c", c=2)[:, None, :, :].broadcast_to(
                (128, NQB, 12, 2)
            ),
            OP.mult,
        )
        nc.vector.tensor_scalar(refpix[...], refpix[...], -0.5, None, OP.add)

        NH = 2  # heads per coord-group
        NQC = NQB // 8  # query blocks per stage-4 chunk
        for hg in range(H // NH):
          for q0 in range(0, NQB, NQC):
            # comparison ALU ops (is_gt/is_ge/...) are DVE-only in walrus
            # codegen, so stage 4 stays on DVE
            tt = nc.vector.tensor_tensor
            tsc = nc.vector.tensor_scalar
            DVE_COPY = nc.vector.tensor_copy
            hs = hg * NH
            shp = (128, NQC, NH, J, 2)
            nel = NQC * NH * J * 2
            s0 = work.tile([128, nel], F32, tag="cs0", bufs=2)
            s2 = work.tile([128, nel], F32, tag="cs2", bufs=2)
            s3 = work.tile([128, nel], F32, tag="cs3", bufs=2)
            s4 = work.tile([128, nel], F32, tag="cs4", bufs=2)
            s5 = work.tile([128, nel], F32, tag="cs5", bufs=2)
            ti = work.tile([128, nel], I32, tag="cti", bufs=2)
            v0 = lambda t: t.rearrange("p (q h j c) -> p q h j c", q=NQC, h=NH, j=J)
            csb = cs.rearrange("p (j c) -> p j c", c=2)[:, None, None, :, :].broadcast_to(shp)
            dmaxb = dmax.rearrange("p (j c) -> p j c", c=2)[:, None, None, :, :].broadcast_to(shp)
            dmaxm1b = dmaxm1.rearrange("p (j c) -> p j c", c=2)[:, None, None, :, :].broadcast_to(shp)
            offv = off_sb.rearrange("p q (h j c) -> p q h j c", h=H, c=2)[:, ds(q0, NQC), ds(hs, NH), :, :]
            tt(v0(s0), offv, csb, OP.mult)
            tt(
                v0(s0), v0(s0),
                refpix.rearrange("p q (j c) -> p q j c", c=2)[:, ds(q0, NQC), None, :, :]
                .broadcast_to(shp),
                OP.add,
            )
            DVE_COPY(ti[...], s0[...])
            DVE_COPY(s2[...], ti[...])
            tt(s3[...], s2[...], s0[...], OP.is_gt)
            tt(s2[...], s2[...], s3[...], OP.subtract)
            tt(s0[...], s0[...], s2[...], OP.subtract)
            tsc(s3[...], s2[...], 0.0, None, OP.is_ge)
            tt(v0(s4), v0(s2), dmaxb, OP.is_le)
            tt(s3[...], s3[...], s4[...], OP.mult)
            tt(v0(s5), v0(s2), dmaxm1b, OP.is_le)
            tsc(s4[...], s2[...], -1.0, None, OP.is_ge)
            tt(s4[...], s4[...], s5[...], OP.mult)
            tsc(s5[...], s0[...], -1.0, 1.0, OP.mult, OP.add)
            tt(s3[...], s5[...], s3[...], OP.mult)
            tt(s4[...], s0[...], s4[...], OP.mult)
            xslice = lambda t: v0(t)[:, :, :, :, 0]
            yslice = lambda t: v0(t)[:, :, :, :, 1]
            wyT = work.tile([128, nel // 2], F32, tag="wyT", bufs=2)
            wyB = work.tile([128, nel // 2], F32, tag="wyB", bufs=2)
            v1 = lambda t: t.rearrange("p (q h j) -> p q h j", q=NQC, h=NH)
            attv = attn[:, ds(q0, NQC), ds(hs, NH), :]
            tt(v1(wyT), yslice(s3), attv, OP.mult)
            tt(v1(wyB), yslice(s4), attv, OP.mult)
            w4v = W4[:, ds(q0, NQC), ds(hs, NH), :, :]
            tt(w4v[:, :, :, :, 0], v1(wyT), xslice(s3), OP.mult)
            tt(w4v[:, :, :, :, 1], v1(wyT), xslice(s4), OP.mult)
            tt(w4v[:, :, :, :, 2], v1(wyB), xslice(s3), OP.mult)
            tt(w4v[:, :, :, :, 3], v1(wyB), xslice(s4), OP.mult)
            tsc(s3[...], s2[...], -1.0, None, OP.max)
            tt(v0(s3), v0(s3), dmaxb, OP.min)
            wmulb = wmul[:, None, None, :].broadcast_to((128, NQC, NH, J))
            lpbb = lpb[:, None, None, :].broadcast_to((128, NQC, NH, J))
            pT = wyT  # reuse buffer
            tt(v1(pT), yslice(s3), wmulb, OP.mult)
            tt(v1(pT), v1(pT), xslice(s3), OP.add)
            tt(v1(pT), v1(pT), lpbb, OP.add)
            for hh in range(NH):
                DVE_COPY(idx[:, hs + hh, ds(q0, NQC), :], v1(pT)[:, :, hh, :])

    A_b = [A, bpool.tile([128, NQB, D], BF16, tag="A1", name="A_1")]

    def fold(b, hp):
        # fold this head pair's indices into wrapped int16
        idx = idx_b[b]
        w16_h = []
        for h2 in range(2):
            h = hp * 2 + h2
            idxw = work.tile([128, 768], I32, tag="idxw", bufs=4)
            w16 = idxw.bitcast(mybir.dt.int16)  # [128, 1536]
            Xh = idx[:, h, :, :].rearrange("p q j -> p (q j)")
            Ysb = work.tile([128, 2, 128], F32, tag="Ysb", bufs=1)
            for c in range(2):
                ptr = psumt.tile([128, 128], F32, tag="tpf")
                nc.tensor.transpose(
                    ptr[:96, :], Xh[:, ds(c * 96, 96)], ident[...]
                )
                ACOPY(Ysb[:96, c, :], ptr[:96, :])
            for qq in range(8):
                for c in range(2):
                    ptr2 = psumt.tile([128, 128], F32, tag="tpf")
                    nc.tensor.transpose(
                        ptr2[:16, :96],
                        Ysb[:96, c, ds(qq * 16, 16)],
                        ident[:96, :96],
                    )
                    # split 96 cols into 2 blocks of 48 (4-qb groups)
                    o = c * 768 + qq
                    ACOPY(w16[0:16, o:o + 377:8], ptr2[:16, 0:48])
                    ACOPY(w16[0:16, o + 384:o + 384 + 377:8],
                          ptr2[:16, 48:96])
            # replicate wrapped block to the other 7 core blocks
            # (log2 doubling: 16->32->64->128 partitions)
            for rep in (16, 32, 64):
                nc.sync.dma_start(
                    w16[ds(rep, rep), :], w16[ds(0, rep), :]
                )
            w16_h.append(w16)
        return w16_h

    def issue_gathers(b, hp, qg, w16_h):
        G = work.tile([128, 2, 48, 128], BF16, tag="G", name="G")
        for h2 in range(2):
            h = hp * 2 + h2
            nc.gpsimd.dma_gather(
                G[:, h2, :, :],
                vp_d[b][h, :, :],
                w16_h[h2][:, ds(qg * 384, 384)],
                num_idxs=4 * J * 128,
                num_idxs_reg=4 * J * 128,
                elem_size=128,
                single_packet=False,
            )
        return G

    def combines(b, hp, qg, G):
        # one mult+reduce per head covering all 4 query blocks of the
        # gather group; engines alternate DVE/Pool per (b, qg, head)
        W4 = W4_b[b]
        for h2 in range(2):
            h = hp * 2 + h2
            eng = nc.gpsimd if (b + qg + h2) % 2 == 0 else nc.vector
            # tag "vacc" reuses stage-3's buffers (dead once combines run)
            Pm = work.tile([128, 4, J, 4, HD], BF16, tag="vacc", name="Pm")
            eng.tensor_tensor(
                Pm[...],
                G[:, h2, :, :].rearrange(
                    "p (q j) (sl e) -> p q j sl e", j=J, e=HD
                ),
                W4[:, ds(qg * 4, 4), h, :, :][:, :, :, :, None]
                .broadcast_to((128, 4, J, 4, HD)),
                OP.mult,
            )
            Ared = work.tile([128, 4, HD], F32, tag="Ared", bufs=2)
            nc.vector.tensor_reduce(
                Ared[...],
                Pm.rearrange("p q j sl e -> p q e j sl"),
                AX.XY, OP.add,
            )
            ACOPY(
                A_b[b][:, ds(qg * 4, 4), ds(h * HD, HD)],
                Ared[...],
            )

    def stage56_pair():
        # ---- stage 5 both batches interleaved: one batch's combine fills
        # the other's gather latency; on the last head pair each gather
        # group's out-projection (stage 6) is emitted as soon as its 4
        # query blocks have all heads combined, hiding it under the
        # remaining gathers ----
        prev = None

        def drain(prev):
            combines(*prev)
            if prev[1] == 3:  # last head pair: those qbs are complete
                stage6_qbs(prev[0], range(prev[2] * 4, prev[2] * 4 + 4))

        for hp in range(4):
            w16_b = [fold(0, hp), fold(1, hp)]
            for qg in range(4):
                for bb in (0, 1):
                    Gn = issue_gathers(bb, hp, qg, w16_b[bb])
                    if prev is not None:
                        drain(prev)
                    prev = (bb, hp, qg, Gn)
        drain(prev)

    def stage6_qbs(b, qbs):
        for qb in qbs:
            pt = psumt.tile([128, 2, 128], BF16, tag="tpb", bufs=1)
            for k in range(2):
                nc.tensor.transpose(
                    pt[:, k, :], A_b[b][:, qb, ds(k * 128, 128)], identb[...]
                )
            AT = work.tile([128, 2, 128], BF16, tag="AT", bufs=1)
            ACOPY(AT[...], pt[...])
            po = psum.tile([128, D], F32, tag="pv")
            for k in range(2):
                nc.tensor.matmul(
                    po[...], AT[:, k, :], wout[:, k, :],
                    start=(k == 0), stop=(k == 1),
                )
            osb = work.tile([128, D], BF16, tag="osb", bufs=1)
            ACOPY(osb[...], po[...])
            nc.sync.dma_start(out_d[b, ds(qb * 128, 128), :], osb[...])

    # issue order: stage12 before stage3 so the projection matmuls aren't
    # queued behind stage 3's ~420 PE instructions (stage 4 DVE work then
    # overlaps stage 3's DMA/PE); b1's prep stages issue after b0's
    # combine so they fill PE/ACT/DMA while b0's gathers+reduces run.
    stage12(0)
    stage3(0); stage12(1); stage4(0)
    stage3(1); stage4(1)
    stage56_pair()
    ctx.close()


_CACHED = None


def _get_program():
    global _CACHED
    if _CACHED is None:
        _CACHED = _build_program()
    return _CACHED


def make_host_inputs(Wq, Wref, Woff, off_bias, Wattn, Wv, Wout):
    """Device-layout weight/constant arrays shared by all cores."""
    bf = ml_dtypes.bfloat16
    wcat = np.concatenate([Woff, Wref, Wattn], axis=1)  # [256, 290]

    def halves(w):
        return np.ascontiguousarray(w.reshape(2, 128, -1).transpose(1, 0, 2))

    rep = lambda v: np.ascontiguousarray(
        np.broadcast_to(np.asarray(v, np.float32)[None, :], (128, len(v)))
    )
    cs24, dmax24, dmaxm1 = [], [], []
    wmul12, lpb12 = [], []
    for l, (hh, ww) in enumerate(SHAPES):
        for p in range(P):
            cs24 += [float(ww), float(hh)]
            dmax24 += [float(ww - 1), float(hh - 1)]
            dmaxm1 += [float(ww - 2), float(hh - 2)]
            wmul12.append(float(ww))
            lpb12.append(float(LVSTART[l]))
    return {
        "wq": halves(Wq).astype(np.float32),
        "wcat": halves(wcat).astype(np.float32),
        "wv": halves(Wv).astype(bf),
        "wout": halves(Wout).astype(bf),
        "bias_rep": rep(off_bias.astype(np.float32)),
        "cs24": rep(cs24),
        "dmax24": rep(dmax24),
        "dmaxm1": rep(dmaxm1),
        "wmul12": rep(wmul12),
        "lpb12": rep(lpb12),
    }


def make_in_maps(x, encoder_input, host_w):
    bf = ml_dtypes.bfloat16
    x = np.asarray(x).astype(np.float16)
    enc = np.asarray(encoder_input).astype(bf)
    in_maps = []
    for c in range(NCORES):
        m = dict(host_w)
        m["x"] = np.ascontiguousarray(x[c * BPC:(c + 1) * BPC])
        m["enc"] = np.ascontiguousarray(enc[c * BPC:(c + 1) * BPC])
        in_maps.append(m)
    return in_maps


def kernel(x, encoder_input, Wq, Wref, Woff, off_bias, Wattn, Wv, Wout):
    from concourse import bass_utils

    nc = _get_program()
    host_w = make_host_inputs(
        np.asarray(Wq, np.float32), np.asarray(Wref, np.float32),
        np.asarray(Woff, np.float32), np.asarray(off_bias, np.float32),
        np.asarray(Wattn, np.float32), np.asarray(Wv, np.float32),
        np.asarray(Wout, np.float32),
    )
    in_maps = make_in_maps(x, encoder_input, host_w)
    res = bass_utils.run_bass_kernel_spmd(nc, in_maps, core_ids=list(range(NCORES)))
    return np.concatenate(
        [np.asarray(r["out"]).astype(np.float32) for r in res.results], axis=0
    )

